# revision 1
# baseline (speedup 1.0000x reference)
"""Adaptive mean thresholding (11x11 box, replicate border, C=0.02) on 8
TRN2 NeuronCores. Batch [128,512,512] f32 -> binary-inv threshold map.

Strategy (pure data parallel, 16 images per core). The DVE is the
critical engine (its scan ucode runs at ~2 cyc/elem, tensor_tensor fp16
at 0.5), so the horizontal 11-tap window is computed at HALF resolution
on the DVE and repaired inside the TensorE pass:

  - ACT deinterleaves the padded f32 image into fp16 even/odd column
    planes (also consumed by the TensorE fix-up pass).
  - DVE: q = even + odd (fp16 tensor_tensor at 2x) = non-overlapping
    pair sums; ONE tensor_tensor_scan per image over the pair domain
    (half the elements of a full-res scan) computes
    R6[m] = sum(q[m..m+5]) + kappa, fp16 out. The scan telescopes
    exactly, so all 4 row-blocks (each [3 pad-pairs | 256 | 3 pad-pairs]
    at uniform 262 stride) ride one recurrence; boundary outputs are
    garbage but never read. kappa = -2.42/11 pre-loads the threshold
    constant so PSUM lands at S2d - 121*0.02 exactly.
  - Window identity: W(2m) = R6[m] - x[2m], W(2m+1) = R6[m+1] - x[2m+13]
    (padded coords), so TensorE accumulates 4 band passes per image into
    per-block [even|odd] PSUM planes: +band @ R6 (two shifts) and
    -band @ x-planes (two slices). All weights are integers (exact fp16).
  - Epilogue: ONE DVE scalar_tensor_tensor straight from PSUM:
    out = (psum * 1/121) is_ge x  -> 1.0/0.0. No ACT threshold pass.
  - Software-pipelined across images; loads on the sync HWDGE queue,
    stores on the gpsimd SWDGE queue (separate rings).
"""

import numpy as np

B, H, W = 128, 512, 512
NCORES = 8
NIMG = B // NCORES          # 16 images per core
P = 128                     # partitions
NB = H // P                 # 4 row blocks per image
K = 11                      # box size
PAD = 6                     # left/right replicate pads per block
BW = PAD + W + PAD          # 524: one padded block
IW = NB * BW                # 2096: one padded image
NQ = BW // 2                # 262 pairs per block
NQI = NB * NQ               # 1048 pairs per image
MQ = W // 2                 # 256 output columns per plane per block
R6L = (NB - 1) * NQ + MQ + 1  # 1043: R6 buffer (sh6[0] = init value R0)
KAPPA = -2.42 / K           # folds "- 121*C" into the scan init
CONST = 0.02

_CACHE = {}


def _band_weights():
    """512x512 vertical box-filter count matrix, sliced to the five
    distinct 128x128 blocks, plus the negated set for the fix-up pass."""
    Bm = np.zeros((H, H), dtype=np.float32)
    for i in range(H):
        for d in range(-5, 6):
            r = min(max(i + d, 0), H - 1)
            Bm[r, i] += 1.0
    W0 = Bm[0:128, 0:128]        # block 0 main (top replicate folded)
    WI = Bm[128:256, 128:256]    # interior main (pure band)
    W3 = Bm[384:512, 384:512]    # block 3 main (bottom replicate folded)
    WDN = Bm[0:128, 128:256]     # contribution from tile I-1 to block I
    WUP = Bm[128:256, 0:128]     # contribution from tile I+1 to block I
    pos = np.stack([W0, WI, W3, WDN, WUP])
    return np.ascontiguousarray(
        np.concatenate([pos, -pos])).astype(np.float16)


def _build():
    import concourse.bass as bass  # noqa: F401
    import concourse.tile as tile
    from concourse import bacc, mybir
    from concourse.alu_op_type import AluOpType

    F32 = mybir.dt.float32
    F16 = mybir.dt.float16
    ACT_COPY = mybir.ActivationFunctionType.Copy

    nc = bacc.Bacc("TRN2", target_bir_lowering=False, debug=False,
                   num_devices=NCORES)
    in_ext = nc.dram_tensor("input", [NIMG, H, W], F32,
                            kind="ExternalInput").ap()
    wts_ext = nc.dram_tensor("wts", [10, 128, 128], F16,
                             kind="ExternalInput").ap()
    out_ext = nc.dram_tensor("output", [NIMG, H, W], F32,
                             kind="ExternalOutput").ap()

    with tile.TileContext(nc) as tc:
        with tc.tile_pool(name="consts", bufs=1) as consts, \
             tc.tile_pool(name="xp", bufs=8) as xp_pool, \
             tc.tile_pool(name="xq", bufs=4) as xq_pool, \
             tc.tile_pool(name="qb", bufs=4) as qb_pool, \
             tc.tile_pool(name="r6", bufs=4) as r6_pool, \
             tc.tile_pool(name="ot", bufs=4) as ot_pool, \
             tc.tile_pool(name="small", bufs=6) as small, \
             tc.tile_pool(name="psum", bufs=2, space="PSUM") as psum:

            wt = consts.tile([P, 10 * 128], F16)
            wv = wt[:].rearrange("p (s m) -> p s m", s=10)
            W0, WI, W3, WDN, WUP = (wv[:, s, :] for s in range(5))
            N0, NI, N3, NDN, NUP = (wv[:, 5 + s, :] for s in range(5))
            MAIN = (W0, WI, WI, W3)
            NMAIN = (N0, NI, NI, N3)
            wts_loaded = []

            def load_wts():
                # on the gpsimd SWDGE queue: keeps the sync HWDGE queue
                # free so the first image load starts at boot
                if not wts_loaded:
                    nc.gpsimd.dma_start(
                        wt[:].rearrange("p (s m) -> p s m", s=10),
                        wts_ext.rearrange("s p m -> p s m"))
                    wts_loaded.append(True)

            live = {}

            def band(ps_slice, blk, srcs, start):
                """Accumulate one band pass (weights W per source block)
                into ps_slice for output block `blk`. srcs[t] is the
                moving tensor of source block t. Returns updated start."""
                for wmat, t in srcs:
                    nc.tensor.matmul(ps_slice, wmat, t,
                                     start=start, stop=False)
                    start = False
                return start

            def front(im):
                xp = xp_pool.tile([P, IW], F32, tag="xp")
                xpv = xp[:].rearrange("p (b c) -> p b c", b=NB)
                src = in_ext[im].rearrange("(b p) w -> p b w", p=P)
                nc.sync.dma_start(xpv[:, :, PAD:PAD + W], src)
                # replicate-pad block edges (ACT)
                nc.scalar.activation(
                    xpv[:, :, 0:PAD],
                    xpv[:, :, PAD:PAD + 1].broadcast_to([P, NB, PAD]),
                    ACT_COPY, bias=0.0, scale=1.0)
                nc.scalar.activation(
                    xpv[:, :, PAD + W:BW],
                    xpv[:, :, PAD + W - 1:PAD + W]
                    .broadcast_to([P, NB, PAD]),
                    ACT_COPY, bias=0.0, scale=1.0)
                # deinterleave to fp16 even/odd column planes (ACT)
                xq = xq_pool.tile([P, 2 * NQI], F16, tag="xq")
                xqe = xq[:, 0:NQI]
                xqo = xq[:, NQI:2 * NQI]
                xpar = xp[:].rearrange("p (b m r) -> p r b m", b=NB, r=2)
                nc.scalar.activation(
                    xqe.rearrange("p (b m) -> p b m", b=NB),
                    xpar[:, 0], ACT_COPY, bias=0.0, scale=1.0)
                nc.scalar.activation(
                    xqo.rearrange("p (b m) -> p b m", b=NB),
                    xpar[:, 1], ACT_COPY, bias=0.0, scale=1.0)
                # pair sums q (DVE fp16 2x)
                qb = qb_pool.tile([P, NQI], F16, tag="qb")
                nc.vector.tensor_tensor(
                    out=qb[:], in0=xqe, in1=xqo, op=AluOpType.add)
                # scan initial R0 = sum(q[0..5]) + kappa, computed straight
                # from the xq planes (keeps it off the q->scan chain), and
                # also written to sh6[0] (the first R6 value)
                init = small.tile([P, 1], F32, tag="init")
                scr = small.tile([P, 2 * 6], F32, tag="scr")
                xq12 = xq[:].rearrange("p (h j) -> p h j", h=2)[:, :, 0:6]
                nc.scalar.activation(
                    scr[:].rearrange("p (h j) -> p h j", h=2), xq12,
                    ACT_COPY, bias=KAPPA / 12.0, scale=1.0,
                    accum_out=init[:])
                sh6 = r6_pool.tile([P, R6L], F16, tag="sh6")
                nc.scalar.activation(sh6[:, 0:1], init[:], ACT_COPY,
                                     bias=0.0, scale=1.0)
                # 6-tap sliding sum over the pair domain (DVE scan):
                # sh6[1+t] = sum(qb[t+1..t+6]) + kappa
                nc.vector.tensor_tensor_scan(
                    out=sh6[:, 1:R6L],
                    data0=qb[:, 6:6 + R6L - 1],
                    data1=qb[:, 0:R6L - 1],
                    initial=init[:],
                    op0=AluOpType.add, op1=AluOpType.subtract)

                # vertical band matmuls: 4 passes into [even|odd] planes
                load_wts()
                xqev = xqe.rearrange("p (b m) -> p b m", b=NB)
                xqov = xqo.rearrange("p (b m) -> p b m", b=NB)

                def r6s(t, sh):   # R6 moving tensor, block t, shift sh
                    base = NQ * t + sh
                    return sh6[:, base:base + MQ]

                def xes(t):       # x even plane, block t
                    return xqev[:, t, 0:MQ]

                def xos(t):       # x odd plane, block t
                    return xqov[:, t, PAD:PAD + MQ]

                ps = psum.tile([P, NB * W], F32, tag="ps")
                for b in range(NB):
                    lo = max(b - 1, 0)
                    hi = min(b + 1, NB - 1)
                    wsel = []
                    for t in range(lo, hi + 1):
                        wmat = (MAIN[b], NMAIN[b]) if t == b else \
                               ((WDN, NDN) if t == b - 1 else (WUP, NUP))
                        wsel.append((t, wmat))
                    for pl in range(2):
                        psl = ps[:, W * b + MQ * pl:W * b + MQ * (pl + 1)]
                        n = len(wsel) * 2
                        i = 0
                        for t, (wp, wn) in wsel:
                            mv = r6s(t, pl)
                            nc.tensor.matmul(psl, wp, mv,
                                             start=(i == 0),
                                             stop=(i == n - 1))
                            i += 1
                        for t, (wp, wn) in wsel:
                            mv = xes(t) if pl == 0 else xos(t)
                            nc.tensor.matmul(psl, wn, mv,
                                             start=(i == 0),
                                             stop=(i == n - 1))
                            i += 1
                live[im] = (xp, ps)

            def cmp_store(im, xp, ps, rng_b, eng):
                """Compare rows blocks rng_b and store them."""
                b0, b1 = rng_b
                nb = b1 - b0
                ot = ot_pool.tile([P, nb * W], F32, tag=f"ot{b0}")
                psv = ps[:].rearrange("p (b r m) -> p b r m",
                                      b=NB, r=2)[:, b0:b1]
                xpv2 = xp[:].rearrange("p (b c) -> p b c", b=NB)[
                    :, b0:b1, PAD:PAD + W].rearrange(
                        "p b (m r) -> p b r m", r=2)
                otv = ot[:].rearrange("p (b c) -> p b c", b=nb).rearrange(
                    "p b (m r) -> p b r m", r=2)
                for pl in range(2):
                    nc.vector.scalar_tensor_tensor(
                        out=otv[:, :, pl], in0=psv[:, :, pl],
                        scalar=1.0 / (K * K), in1=xpv2[:, :, pl],
                        op0=AluOpType.mult, op1=AluOpType.is_ge)
                dst = out_ext[im].rearrange("(b p) w -> p b w", p=P)
                eng.dma_start(
                    dst[:, b0:b1],
                    ot[:].rearrange("p (b c) -> p b c", b=nb))

            def epilogue(im):
                xp, ps = live.pop(im)
                if im < NIMG - 2:
                    cmp_store(im, xp, ps, (0, NB), nc.gpsimd)
                else:
                    # drain tail at half-image granularity; stores on the
                    # sync queue so the gpsimd dge-drain isn't the tail
                    cmp_store(im, xp, ps, (0, 2), nc.sync)
                    cmp_store(im, xp, ps, (2, 4), nc.sync)

            for im in range(NIMG):
                front(im)
                if im >= 1:
                    epilogue(im - 1)
            epilogue(NIMG - 1)

    nc.compile()
    return nc


def _get_nc():
    if "nc" not in _CACHE:
        _CACHE["nc"] = _build()
        _CACHE["wts"] = _band_weights()
    return _CACHE["nc"]


def kernel(input_batch: np.ndarray) -> np.ndarray:
    from concourse.bass_utils import run_bass_kernel_spmd

    nc = _get_nc()
    wts = _CACHE["wts"]
    assert input_batch.shape == (B, H, W)
    x = np.ascontiguousarray(input_batch, dtype=np.float32)
    in_maps = [
        {"input": x[c * NIMG:(c + 1) * NIMG], "wts": wts}
        for c in range(NCORES)
    ]
    res = run_bass_kernel_spmd(nc, in_maps, core_ids=list(range(NCORES)))
    return np.concatenate([r["output"] for r in res.results], axis=0)


if __name__ == "__main__":
    rng = np.random.default_rng(0)
    x = rng.random((B, H, W), dtype=np.float32)
    y = kernel(x)
    print(y.shape, y.dtype, y.mean())



# revision 4
# speedup vs baseline: 1.2271x; 1.2271x over previous
"""Adaptive mean thresholding (11x11 box, replicate border, C=0.02) on 8
TRN2 NeuronCores. Batch [128,512,512] f32 -> binary-inv threshold map.

Strategy (pure data parallel, 16 images per core). The DVE is the
critical engine (its scan ucode runs at ~2 cyc/elem, tensor_tensor fp16
at 0.5), so the horizontal 11-tap window is computed at HALF resolution
on the DVE and repaired inside the TensorE pass:

  - ACT deinterleaves the padded f32 image into fp16 even/odd column
    planes (also consumed by the TensorE fix-up pass).
  - DVE: q = even + odd (fp16 tensor_tensor at 2x) = non-overlapping
    pair sums; ONE tensor_tensor_scan per image over the pair domain
    (half the elements of a full-res scan) computes
    R6[m] = sum(q[m..m+5]) + kappa, fp16 out. The scan telescopes
    exactly, so all 4 row-blocks (each [3 pad-pairs | 256 | 3 pad-pairs]
    at uniform 262 stride) ride one recurrence; boundary outputs are
    garbage but never read. kappa = -2.42/11 pre-loads the threshold
    constant so PSUM lands at S2d - 121*0.02 exactly.
  - Window identity: W(2m) = R6[m] - x[2m], W(2m+1) = R6[m+1] - x[2m+13]
    (padded coords), so TensorE accumulates 4 band passes per image into
    per-block [even|odd] PSUM planes: +band @ R6 (two shifts) and
    -band @ x-planes (two slices). All weights are integers (exact fp16).
  - Epilogue: ONE DVE scalar_tensor_tensor straight from PSUM:
    out = (psum * 1/121) is_ge x  -> 1.0/0.0. No ACT threshold pass.
  - Software-pipelined across images; loads on the sync HWDGE queue,
    stores on the gpsimd SWDGE queue (separate rings).
"""

import numpy as np

B, H, W = 128, 512, 512
NCORES = 8
NIMG = B // NCORES          # 16 images per core
P = 128                     # partitions
NB = H // P                 # 4 row blocks per image
K = 11                      # box size
PAD = 6                     # left/right replicate pads per block
BW = PAD + W + PAD          # 524: one padded block
IW = NB * BW                # 2096: one padded image
NQ = BW // 2                # 262 pairs per block
NQI = NB * NQ               # 1048 pairs per image
MQ = W // 2                 # 256 output columns per plane per block
R6L = (NB - 1) * NQ + MQ + 1  # 1043: R6 buffer (sh6[0] = init value R0)
KAPPA = -2.42 / K           # folds "- 121*C" into the scan init
CONST = 0.02

_CACHE = {}


def _band_weights():
    """512x512 vertical box-filter count matrix, sliced to the five
    distinct 128x128 blocks, plus the negated set for the fix-up pass."""
    Bm = np.zeros((H, H), dtype=np.float32)
    for i in range(H):
        for d in range(-5, 6):
            r = min(max(i + d, 0), H - 1)
            Bm[r, i] += 1.0
    W0 = Bm[0:128, 0:128]        # block 0 main (top replicate folded)
    WI = Bm[128:256, 128:256]    # interior main (pure band)
    W3 = Bm[384:512, 384:512]    # block 3 main (bottom replicate folded)
    WDN = Bm[0:128, 128:256]     # contribution from tile I-1 to block I
    WUP = Bm[128:256, 0:128]     # contribution from tile I+1 to block I
    pos = np.stack([W0, WI, W3, WDN, WUP])
    return np.ascontiguousarray(
        np.concatenate([pos, -pos])).astype(np.float16)


def _build():
    import concourse.bass as bass  # noqa: F401
    import concourse.tile as tile
    from concourse import bacc, mybir
    from concourse.alu_op_type import AluOpType

    F32 = mybir.dt.float32
    F16 = mybir.dt.float16
    U8 = mybir.dt.uint8
    ACT_COPY = mybir.ActivationFunctionType.Copy

    nc = bacc.Bacc("TRN2", target_bir_lowering=False, debug=False,
                   num_devices=NCORES)
    in_ext = nc.dram_tensor("input", [NIMG, H, W], F32,
                            kind="ExternalInput").ap()
    wts_ext = nc.dram_tensor("wts", [10, 128, 128], F16,
                             kind="ExternalInput").ap()
    out_ext = nc.dram_tensor("output", [NIMG, H, W], U8,
                             kind="ExternalOutput").ap()

    with tile.TileContext(nc) as tc:
        with tc.tile_pool(name="consts", bufs=1) as consts, \
             tc.tile_pool(name="xp", bufs=8) as xp_pool, \
             tc.tile_pool(name="xq", bufs=4) as xq_pool, \
             tc.tile_pool(name="qb", bufs=4) as qb_pool, \
             tc.tile_pool(name="r6", bufs=4) as r6_pool, \
             tc.tile_pool(name="ot", bufs=4) as ot_pool, \
             tc.tile_pool(name="small", bufs=6) as small, \
             tc.tile_pool(name="psum", bufs=2, space="PSUM") as psum:

            wt = consts.tile([P, 10 * 128], F16)
            wv = wt[:].rearrange("p (s m) -> p s m", s=10)
            W0, WI, W3, WDN, WUP = (wv[:, s, :] for s in range(5))
            N0, NI, N3, NDN, NUP = (wv[:, 5 + s, :] for s in range(5))
            MAIN = (W0, WI, WI, W3)
            NMAIN = (N0, NI, NI, N3)
            wts_loaded = []

            def load_wts():
                # on the gpsimd SWDGE queue: keeps the sync HWDGE queue
                # free so the first image load starts at boot
                if not wts_loaded:
                    nc.gpsimd.dma_start(
                        wt[:].rearrange("p (s m) -> p s m", s=10),
                        wts_ext.rearrange("s p m -> p s m"))
                    wts_loaded.append(True)

            live = {}

            def band(ps_slice, blk, srcs, start):
                """Accumulate one band pass (weights W per source block)
                into ps_slice for output block `blk`. srcs[t] is the
                moving tensor of source block t. Returns updated start."""
                for wmat, t in srcs:
                    nc.tensor.matmul(ps_slice, wmat, t,
                                     start=start, stop=False)
                    start = False
                return start

            def front(im):
                xp = xp_pool.tile([P, IW], F32, tag="xp")
                xpv = xp[:].rearrange("p (b c) -> p b c", b=NB)
                src = in_ext[im].rearrange("(b p) w -> p b w", p=P)
                nc.sync.dma_start(xpv[:, :, PAD:PAD + W], src)
                # replicate-pad block edges (ACT)
                nc.scalar.activation(
                    xpv[:, :, 0:PAD],
                    xpv[:, :, PAD:PAD + 1].broadcast_to([P, NB, PAD]),
                    ACT_COPY, bias=0.0, scale=1.0)
                nc.scalar.activation(
                    xpv[:, :, PAD + W:BW],
                    xpv[:, :, PAD + W - 1:PAD + W]
                    .broadcast_to([P, NB, PAD]),
                    ACT_COPY, bias=0.0, scale=1.0)
                # deinterleave to fp16 even/odd column planes (ACT)
                xq = xq_pool.tile([P, 2 * NQI], F16, tag="xq")
                xqe = xq[:, 0:NQI]
                xqo = xq[:, NQI:2 * NQI]
                xpar = xp[:].rearrange("p (b m r) -> p r b m", b=NB, r=2)
                nc.scalar.activation(
                    xqe.rearrange("p (b m) -> p b m", b=NB),
                    xpar[:, 0], ACT_COPY, bias=0.0, scale=1.0)
                nc.scalar.activation(
                    xqo.rearrange("p (b m) -> p b m", b=NB),
                    xpar[:, 1], ACT_COPY, bias=0.0, scale=1.0)
                # pair sums q (DVE fp16 2x)
                qb = qb_pool.tile([P, NQI], F16, tag="qb")
                nc.vector.tensor_tensor(
                    out=qb[:], in0=xqe, in1=xqo, op=AluOpType.add)
                # scan initial R0 = sum(q[0..5]) + kappa, computed straight
                # from the xq planes (keeps it off the q->scan chain), and
                # also written to sh6[0] (the first R6 value)
                init = small.tile([P, 1], F32, tag="init")
                scr = small.tile([P, 2 * 6], F32, tag="scr")
                xq12 = xq[:].rearrange("p (h j) -> p h j", h=2)[:, :, 0:6]
                nc.scalar.activation(
                    scr[:].rearrange("p (h j) -> p h j", h=2), xq12,
                    ACT_COPY, bias=KAPPA / 12.0, scale=1.0,
                    accum_out=init[:])
                sh6 = r6_pool.tile([P, R6L], F16, tag="sh6")
                nc.scalar.activation(sh6[:, 0:1], init[:], ACT_COPY,
                                     bias=0.0, scale=1.0)
                # 6-tap sliding sum over the pair domain (DVE scan):
                # sh6[1+t] = sum(qb[t+1..t+6]) + kappa
                nc.vector.tensor_tensor_scan(
                    out=sh6[:, 1:R6L],
                    data0=qb[:, 6:6 + R6L - 1],
                    data1=qb[:, 0:R6L - 1],
                    initial=init[:],
                    op0=AluOpType.add, op1=AluOpType.subtract)

                # vertical band matmuls: 4 passes into [even|odd] planes
                load_wts()
                xqev = xqe.rearrange("p (b m) -> p b m", b=NB)
                xqov = xqo.rearrange("p (b m) -> p b m", b=NB)

                def r6s(t, sh):   # R6 moving tensor, block t, shift sh
                    base = NQ * t + sh
                    return sh6[:, base:base + MQ]

                def xes(t):       # x even plane, block t
                    return xqev[:, t, 0:MQ]

                def xos(t):       # x odd plane, block t
                    return xqov[:, t, PAD:PAD + MQ]

                ps = psum.tile([P, NB * W], F32, tag="ps")
                for b in range(NB):
                    lo = max(b - 1, 0)
                    hi = min(b + 1, NB - 1)
                    wsel = []
                    for t in range(lo, hi + 1):
                        wmat = (MAIN[b], NMAIN[b]) if t == b else \
                               ((WDN, NDN) if t == b - 1 else (WUP, NUP))
                        wsel.append((t, wmat))
                    for pl in range(2):
                        psl = ps[:, W * b + MQ * pl:W * b + MQ * (pl + 1)]
                        n = len(wsel) * 2
                        i = 0
                        for t, (wp, wn) in wsel:
                            mv = r6s(t, pl)
                            nc.tensor.matmul(psl, wp, mv,
                                             start=(i == 0),
                                             stop=(i == n - 1))
                            i += 1
                        for t, (wp, wn) in wsel:
                            mv = xes(t) if pl == 0 else xos(t)
                            nc.tensor.matmul(psl, wn, mv,
                                             start=(i == 0),
                                             stop=(i == n - 1))
                            i += 1
                live[im] = (xp, ps)

            def cmp_store(im, xp, ps, rng_b, eng):
                """Compare rows blocks rng_b and store them (u8 out,
                plane-major per row: [evens(256) | odds(256)]; the host
                re-interleaves)."""
                b0, b1 = rng_b
                nb = b1 - b0
                ot = ot_pool.tile([P, nb * W], U8, tag=f"ot{b0}")
                psv = ps[:].rearrange("p (b r m) -> p b r m",
                                      b=NB, r=2)[:, b0:b1]
                xpv2 = xp[:].rearrange("p (b c) -> p b c", b=NB)[
                    :, b0:b1, PAD:PAD + W].rearrange(
                        "p b (m r) -> p b r m", r=2)
                otv = ot[:].rearrange("p (b r m) -> p b r m", b=nb, r=2)
                for pl in range(2):
                    nc.vector.scalar_tensor_tensor(
                        out=otv[:, :, pl], in0=psv[:, :, pl],
                        scalar=1.0 / (K * K), in1=xpv2[:, :, pl],
                        op0=AluOpType.mult, op1=AluOpType.is_ge)
                dst = out_ext[im].rearrange("(b p) w -> p b w", p=P)
                eng.dma_start(
                    dst[:, b0:b1],
                    ot[:].rearrange("p (b c) -> p b c", b=nb))

            def epilogue(im):
                xp, ps = live.pop(im)
                if im < NIMG - 2:
                    cmp_store(im, xp, ps, (0, NB), nc.gpsimd)
                else:
                    # drain tail at half-image granularity; stores on the
                    # sync queue so the gpsimd dge-drain isn't the tail
                    cmp_store(im, xp, ps, (0, 2), nc.sync)
                    cmp_store(im, xp, ps, (2, 4), nc.sync)

            for im in range(NIMG):
                front(im)
                if im >= 1:
                    epilogue(im - 1)
            epilogue(NIMG - 1)

    nc.compile()
    return nc


def _get_nc():
    if "nc" not in _CACHE:
        _CACHE["nc"] = _build()
        _CACHE["wts"] = _band_weights()
    return _CACHE["nc"]


def postprocess(outputs) -> np.ndarray:
    """Device output is u8 with each row stored plane-major
    [evens(256) | odds(256)]; re-interleave and cast to f32."""
    raw = np.concatenate(list(outputs), axis=0)          # [B,H,W] u8
    v = raw.reshape(B, H, 2, W // 2)
    out = np.empty((B, H, W), dtype=np.float32)
    out[:, :, 0::2] = v[:, :, 0]
    out[:, :, 1::2] = v[:, :, 1]
    return out


def kernel(input_batch: np.ndarray) -> np.ndarray:
    from concourse.bass_utils import run_bass_kernel_spmd

    nc = _get_nc()
    wts = _CACHE["wts"]
    assert input_batch.shape == (B, H, W)
    x = np.ascontiguousarray(input_batch, dtype=np.float32)
    in_maps = [
        {"input": x[c * NIMG:(c + 1) * NIMG], "wts": wts}
        for c in range(NCORES)
    ]
    res = run_bass_kernel_spmd(nc, in_maps, core_ids=list(range(NCORES)))
    return postprocess(r["output"] for r in res.results)


if __name__ == "__main__":
    rng = np.random.default_rng(0)
    x = rng.random((B, H, W), dtype=np.float32)
    y = kernel(x)
    print(y.shape, y.dtype, y.mean())



# revision 5
# speedup vs baseline: 1.2334x; 1.0052x over previous
"""Adaptive mean thresholding (11x11 box, replicate border, C=0.02) on 8
TRN2 NeuronCores. Batch [128,512,512] f32 -> binary-inv threshold map.

v2.1 design: balance DVE/ACT/PE/GpSimd/DMA near the DMA roofline.

  - u8 output (4x store-traffic cut), plane-major per row
    [evens(256)|odds(256)]; host re-interleaves + casts to f32. Stores
    batched 4 images -> 1 MB SWDGE transfers.
  - DVE: pair sums qb (fp16 2x), ONE scan per image extended 6 steps so
    the initial value is just KAPPA (no ACT-side init/seed ops; qb tile
    has a gpsimd-zeroed 6-elem prefix), and the overcount presubtract
    W = R6 - x as two FULL-WIDTH contiguous fp16 2x tensor_tensors
    (6 junk elems per block are computed and never read - contiguity
    keeps the 2x packed mode, unlike block-strided 3D views).
  - TensorE: positive band passes on W only (20 MM) + the -121*I fold
    of the compare operand x into PSUM (8 MM): z = box - 121C - 121x.
  - ACT: deinterleave x into fp16 even/odd planes + the entire compare:
    u8 out = Sign(z) straight from PSUM (f32->u8 saturates -1 -> 0).
  - GpSimd: replicate pads, qb zero-prefix memset, batched store issue.
"""

import numpy as np

B, H, W = 128, 512, 512
NCORES = 8
NIMG = B // NCORES          # 16 images per core
P = 128                     # partitions
NB = H // P                 # 4 row blocks per image
K = 11                      # box size
PAD = 6                     # left/right replicate pads per block
BW = PAD + W + PAD          # 524: one padded block
IW = NB * BW                # 2096: one padded image
NQ = BW // 2                # 262 pairs per block
NQI = NB * NQ               # 1048 pairs per image
MQ = W // 2                 # 256 output columns per plane per block
KAPPA = -2.42 / K           # folds "- 121*C" into the scan init

_CACHE = {}


def _band_weights():
    """512x512 vertical box-filter count matrix, sliced to the five
    distinct 128x128 blocks, plus the -121*I stationary for the x fold."""
    Bm = np.zeros((H, H), dtype=np.float32)
    for i in range(H):
        for d in range(-5, 6):
            r = min(max(i + d, 0), H - 1)
            Bm[r, i] += 1.0
    W0 = Bm[0:128, 0:128]        # block 0 main (top replicate folded)
    WI = Bm[128:256, 128:256]    # interior main (pure band)
    W3 = Bm[384:512, 384:512]    # block 3 main (bottom replicate folded)
    WDN = Bm[0:128, 128:256]     # contribution from tile I-1 to block I
    WUP = Bm[128:256, 0:128]     # contribution from tile I+1 to block I
    DIAG = -121.0 * np.eye(128, dtype=np.float32)
    return np.ascontiguousarray(
        np.stack([W0, WI, W3, WDN, WUP, DIAG])).astype(np.float16)


def _build():
    import concourse.bass as bass  # noqa: F401
    import concourse.tile as tile
    from concourse import bacc, mybir
    from concourse.alu_op_type import AluOpType

    F32 = mybir.dt.float32
    F16 = mybir.dt.float16
    U8 = mybir.dt.uint8
    ACT_COPY = mybir.ActivationFunctionType.Copy
    ACT_SIGN = mybir.ActivationFunctionType.Sign

    nc = bacc.Bacc("TRN2", target_bir_lowering=False, debug=False,
                   num_devices=NCORES)
    in_ext = nc.dram_tensor("input", [NIMG, H, W], F32,
                            kind="ExternalInput").ap()
    wts_ext = nc.dram_tensor("wts", [6, 128, 128], F16,
                             kind="ExternalInput").ap()
    out_ext = nc.dram_tensor("output", [NIMG, H, W], U8,
                             kind="ExternalOutput").ap()

    with tile.TileContext(nc) as tc:
        with tc.tile_pool(name="consts", bufs=1) as consts, \
             tc.tile_pool(name="xp", bufs=4) as xp_pool, \
             tc.tile_pool(name="xq", bufs=4) as xq_pool, \
             tc.tile_pool(name="qb", bufs=4) as qb_pool, \
             tc.tile_pool(name="r6", bufs=4) as r6_pool, \
             tc.tile_pool(name="wq", bufs=4) as wq_pool, \
             tc.tile_pool(name="ot", bufs=2) as ot_pool, \
             tc.tile_pool(name="psum", bufs=2, space="PSUM") as psum:

            wt = consts.tile([P, 6 * 128], F16)
            wv = wt[:].rearrange("p (s m) -> p s m", s=6)
            W0, WI, W3, WDN, WUP, DIAG = (wv[:, s, :] for s in range(6))
            MAIN = (W0, WI, WI, W3)
            wts_loaded = []

            def load_wts():
                if not wts_loaded:
                    nc.gpsimd.dma_start(
                        wt[:].rearrange("p (s m) -> p s m", s=6),
                        wts_ext.rearrange("s p m -> p s m"))
                    wts_loaded.append(True)

            live = {}
            otbufs = {}

            def front(im):
                xp = xp_pool.tile([P, IW], F32, tag="xp")
                xpv = xp[:].rearrange("p (b c) -> p b c", b=NB)
                src = in_ext[im].rearrange("(b p) w -> p b w", p=P)
                nc.sync.dma_start(xpv[:, :, PAD:PAD + W], src)
                # replicate-pad block edges (GpSimd; tiny broadcast copies)
                nc.gpsimd.tensor_copy(
                    xpv[:, :, 0:PAD],
                    xpv[:, :, PAD:PAD + 1].broadcast_to([P, NB, PAD]))
                nc.gpsimd.tensor_copy(
                    xpv[:, :, PAD + W:BW],
                    xpv[:, :, PAD + W - 1:PAD + W]
                    .broadcast_to([P, NB, PAD]))
                # deinterleave to fp16 even/odd planes (ACT)
                # (+6 tail pad so full-width shifted views stay in range)
                xq = xq_pool.tile([P, 2 * NQI + PAD], F16, tag="xq")
                xqe = xq[:, 0:NQI]
                xqo = xq[:, NQI:2 * NQI]
                xpar = xp[:].rearrange("p (b m r) -> p r b m", b=NB, r=2)
                nc.scalar.activation(
                    xq[:, 0:2 * NQI].rearrange(
                        "p (r b m) -> p r b m", r=2, b=NB),
                    xpar, ACT_COPY, bias=0.0, scale=1.0)
                # pair sums qb (DVE fp16 2x) into a zero-prefixed buffer
                qbz = qb_pool.tile([P, PAD + NQI], F16, tag="qb")
                nc.gpsimd.memset(qbz[:, 0:PAD], 0)
                nc.vector.tensor_tensor(
                    out=qbz[:, PAD:], in0=xqe, in1=xqo, op=AluOpType.add)
                # 6-tap sliding sum over the pair domain (DVE scan),
                # extended 6 steps so initial is just KAPPA:
                #   sh6n[t] = sum(qb[t-5..t]) + KAPPA   (valid for t >= 5)
                # old-style R6 starting at pair m == sh6n[m+5].
                sh6 = r6_pool.tile([P, NQI + PAD], F16, tag="sh6")
                nc.vector.tensor_tensor_scan(
                    out=sh6[:, 0:NQI],
                    data0=qbz[:, PAD:PAD + NQI],
                    data1=qbz[:, 0:NQI],
                    initial=KAPPA,
                    op0=AluOpType.add, op1=AluOpType.subtract)

                # presubtract (DVE fp16 2x, full-width contiguous):
                #   we[j] = sh6n[j+5] - xqe[j]   (= R6[j] - x_even[j])
                #   wo[j] = sh6n[j+6] - xqo[j+6] (= R6[j+1] - x_odd[j+6])
                # junk at j%262 >= 256 is never consumed by the matmuls.
                wq = wq_pool.tile([P, 2 * NQI], F16, tag="wq")
                we = wq[:, 0:NQI]
                wo = wq[:, NQI:2 * NQI]
                nc.vector.tensor_tensor(
                    out=we, in0=sh6[:, 5:5 + NQI], in1=xq[:, 0:NQI],
                    op=AluOpType.subtract)
                nc.vector.tensor_tensor(
                    out=wo, in0=sh6[:, 6:6 + NQI],
                    in1=xq[:, NQI + PAD:2 * NQI + PAD],
                    op=AluOpType.subtract)

                # vertical band matmuls: positive band on W planes, then
                # -121*I on the compare-x slices so PSUM holds
                # z = box - 121*C - 121*x
                load_wts()
                ps = psum.tile([P, NB * W], F32, tag="ps")
                for b in range(NB):
                    lo = max(b - 1, 0)
                    hi = min(b + 1, NB - 1)
                    srcs = []
                    for t in range(lo, hi + 1):
                        wmat = MAIN[b] if t == b else \
                               (WDN if t == b - 1 else WUP)
                        srcs.append((t, wmat))
                    for pl in range(2):
                        psl = ps[:, W * b + MQ * pl:W * b + MQ * (pl + 1)]
                        wpl = we if pl == 0 else wo
                        i = 0
                        for t, wmat in srcs:
                            nc.tensor.matmul(psl, wmat,
                                             wpl[:, NQ * t:NQ * t + MQ],
                                             start=(i == 0), stop=False)
                            i += 1
                        xoff = (0 if pl == 0 else NQI) + NQ * b + 3
                        nc.tensor.matmul(psl, DIAG,
                                         xq[:, xoff:xoff + MQ],
                                         start=False, stop=True)
                live[im] = (ps,)

            def compare(im, ps, ot, slot):
                """Sign(z) -> u8 into slot of a 4-image store buffer."""
                nc.scalar.activation(
                    ot[:, slot * NB * W:(slot + 1) * NB * W],
                    ps[:], ACT_SIGN, bias=0.0, scale=1.0)

            def store_batch(i0, n, eng):
                ot = otbufs.pop(i0)
                dst = out_ext[i0:i0 + n].rearrange(
                    "i (b p) w -> p i b w", p=P)
                eng.dma_start(
                    dst, ot[:, 0:n * NB * W].rearrange(
                        "p (i b w) -> p i b w", i=n, b=NB))

            def epilogue(im):
                (ps,) = live.pop(im)
                if im < 12:
                    i0 = im - im % 4
                    if im % 4 == 0:
                        otbufs[i0] = ot_pool.tile(
                            [P, 4 * NB * W], U8, tag="ot4", name="ot4")
                    compare(im, ps, otbufs[i0], im % 4)
                    if im % 4 == 3:
                        store_batch(i0, 4, nc.gpsimd)
                else:
                    # tail: per-image stores to keep the drain short
                    otbufs[im] = ot_pool.tile(
                        [P, 4 * NB * W], U8, tag="ot4", name="ot1")
                    compare(im, ps, otbufs[im], 0)
                    store_batch(im, 1,
                                nc.gpsimd if im < 14 else nc.sync)

            for im in range(NIMG):
                front(im)
                if im >= 1:
                    epilogue(im - 1)
            epilogue(NIMG - 1)

    nc.compile()
    return nc


def _get_nc():
    if "nc" not in _CACHE:
        _CACHE["nc"] = _build()
        _CACHE["wts"] = _band_weights()
    return _CACHE["nc"]


def postprocess(outputs) -> np.ndarray:
    """Device output is u8 with each row stored plane-major
    [evens(256) | odds(256)]; re-interleave and cast to f32."""
    raw = np.concatenate(list(outputs), axis=0)          # [B,H,W] u8
    v = raw.reshape(B, H, 2, W // 2)
    out = np.empty((B, H, W), dtype=np.float32)
    out[:, :, 0::2] = v[:, :, 0]
    out[:, :, 1::2] = v[:, :, 1]
    return out


def kernel(input_batch: np.ndarray) -> np.ndarray:
    from concourse.bass_utils import run_bass_kernel_spmd

    nc = _get_nc()
    wts = _CACHE["wts"]
    assert input_batch.shape == (B, H, W)
    x = np.ascontiguousarray(input_batch, dtype=np.float32)
    in_maps = [
        {"input": x[c * NIMG:(c + 1) * NIMG], "wts": wts}
        for c in range(NCORES)
    ]
    res = run_bass_kernel_spmd(nc, in_maps, core_ids=list(range(NCORES)))
    return postprocess(r["output"] for r in res.results)


if __name__ == "__main__":
    rng = np.random.default_rng(0)
    x = rng.random((B, H, W), dtype=np.float32)
    y = kernel(x)
    print(y.shape, y.dtype, y.mean())
